# revision 17
# baseline (speedup 1.0000x reference)
"""KQEnergyBlock Trainium2 Bass kernel, v2.

Math per batch element (see reference):
  Q = x Wq^T, K = x Wk^T            (N, D), heads = 64-col slices
  S_h = beta_h Q_h K_h^T ; A_h = softmax(S_h)
  T1 = AVc @ Wq   (AVc  = concat_h A_h K_h)
  T2 = ATQc @ Wk  (ATQc = concat_h A_h^T Q_h)
  out = T1 + T2 + relu(x Wm^T) Wm

Cost-model-driven design (TimelineSim): a matmul instruction costs only its
output free size, so every attention product keeps 128 output partitions and
transposes are offloaded to the DMA XBAR (dma_start(transpose=True)):
  - QT/KT via matmul; Qn/Kn derived from them by DMA transpose.
  - E = exp(beta*S) on ACT (scale=beta, accum_out=rowsum halves);
    A = E * (1/r) in place on DVE; A^T via one DMA transpose per head.
  - AVn[q,z] = sum_k A^T[k,q]^T Kn[k,z], ATQn[k,z] = sum_q A[q,k]^T Qn[q,z];
    head pairs accumulate into one [P,NC,2Z] psum tile, evicted with a single
    contiguous copy, then DMA-transposed into AVT/ATQT.
  - stage4: out = AVc@Wq + ATQc@Wk + hid@Wm accumulated in PSUM.

XBAR transpose dest constraints (probed): contiguous dest or mid-stride a
multiple of 256 bytes; in [128, F] -> out[po, mid, fo] = in[fo, mid*128+po].

Sharding: data-parallel over batch B=8, one element per core, no
collectives.
"""

import numpy as np
import ml_dtypes

import concourse.mybir as mybir
import concourse.tile as tile
from concourse import bacc
from concourse.bass_utils import run_bass_kernel_spmd

B, N, D = 8, 1024, 768
H, Z = 12, 64
HID = 3072
P = 128
DC = D // P     # 6
NC = N // P     # 8
BF = mybir.dt.bfloat16
F32 = mybir.dt.float32
Exp = mybir.ActivationFunctionType.Exp
Add = mybir.AluOpType.add

NPBF = ml_dtypes.bfloat16

_CACHE = {}


def _build(dbg=False):
    nc = bacc.Bacc("TRN2", target_bir_lowering=False, debug=False, num_devices=8)
    dbg_d = {}
    if dbg:
        for nm, shp in (("Qn", [P, NC, D]), ("Kn", [P, NC, D]),
                        ("ET0", [P, NC * NC, P]), ("E0", [P, NC, N]),
                        ("AVT", [P, DC, N]), ("ATQT", [P, DC, N]),
                        ("hid0", [P, N]), ("QT", [P, DC, N])):
            dbg_d[nm] = nc.dram_tensor("dbg_" + nm, shp, BF,
                                       kind="ExternalOutput")
    xT_d = nc.dram_tensor("xT", [D, N], BF, kind="ExternalInput")
    wqT_d = nc.dram_tensor("wqT", [D, D], BF, kind="ExternalInput")
    wkT_d = nc.dram_tensor("wkT", [D, D], BF, kind="ExternalInput")
    wq_d = nc.dram_tensor("wq", [D, D], BF, kind="ExternalInput")
    wk_d = nc.dram_tensor("wk", [D, D], BF, kind="ExternalInput")
    wmT_d = nc.dram_tensor("wmT", [D, HID], BF, kind="ExternalInput")
    wm_d = nc.dram_tensor("wm", [HID, D], BF, kind="ExternalInput")
    betav_d = nc.dram_tensor("betav", [P, H], F32, kind="ExternalInput")
    ident_d = nc.dram_tensor("ident", [P, P], BF, kind="ExternalInput")
    out_d = nc.dram_tensor("out", [N, D], BF, kind="ExternalOutput")

    xT_v = xT_d.ap().rearrange("(c p) n -> p c n", p=P)      # [128, 6, 1024]
    wqT_v = wqT_d.ap().rearrange("(c p) e -> p c e", p=P)
    wkT_v = wkT_d.ap().rearrange("(c p) e -> p c e", p=P)
    wq_v = wq_d.ap().rearrange("(c p) d -> p c d", p=P)
    wk_v = wk_d.ap().rearrange("(c p) d -> p c d", p=P)
    wmT_v = wmT_d.ap().rearrange("(c p) h -> p c h", p=P)    # [128, 6, 3072]
    wm_v = wm_d.ap().rearrange("(c p) d -> p c d", p=P)      # [128, 24, 768]
    out_v = out_d.ap().rearrange("(c p) d -> p c d", p=P)    # [128, 8, 768]

    with tile.TileContext(nc) as tc:
        with (
            tc.tile_pool(name="acts", bufs=1) as acts,
            tc.tile_pool(name="hd", bufs=1) as hd,
            tc.tile_pool(name="stream", bufs=3) as stream,
            tc.tile_pool(name="ps", bufs=2, space="PSUM") as ps,
            tc.tile_pool(name="dram", bufs=1, space="DRAM") as dram,
        ):
            # ---- persistent inputs (ordered so QT's first matmul can start
            # as soon as wqT + the first xT half arrive) ----
            xT = acts.tile([P, DC, N], BF)
            wqT = acts.tile([P, DC, D], BF)
            wkT = acts.tile([P, DC, D], BF)
            wq = acts.tile([P, DC, D], BF)
            wk = acts.tile([P, DC, D], BF)
            betav = acts.tile([P, H], F32)
            ident = acts.tile([P, P], BF)
            nc.sync.dma_start(wqT[:, :, 0:P], wqT_v[:, :, 0:P])
            nc.sync.dma_start(xT[:, :, 0:256], xT_v[:, :, 0:256])
            nc.sync.dma_start(wkT[:, :, 0:P], wkT_v[:, :, 0:P])
            nc.sync.dma_start(xT[:, :, 256:512], xT_v[:, :, 256:512])
            nc.sync.dma_start(xT[:, :, 512:N], xT_v[:, :, 512:N])
            nc.sync.dma_start(wqT[:, :, P:D], wqT_v[:, :, P:D])
            nc.sync.dma_start(wkT[:, :, P:D], wkT_v[:, :, P:D])
            nc.sync.dma_start(betav[:], betav_d.ap())
            nc.sync.dma_start(ident[:], ident_d.ap())
            nc.sync.dma_start(wq[:], wq_v)
            nc.sync.dma_start(wk[:], wk_v)

            QT = acts.tile([P, DC, N], BF)
            KT = acts.tile([P, DC, N], BF)
            Qn = acts.tile([P, NC, D], BF)
            Kn = acts.tile([P, NC, D], BF)
            # ET keeps its padded strided-3D XBAR dest (144 = 128+16)
            PT = P + 16
            # AVT/ATQT are assembled by PE transposes (identity matmuls)
            # into bf16-bitcast PSUM, evicted contiguously by DVE — this
            # keeps the saturated DMA device free for the A^T XBAR.
            AVTs = [acts.tile([P, NC, P], BF, name=f"AVT{c}")
                    for c in range(DC)]
            ATQTs = [acts.tile([P, NC, P], BF, name=f"ATQT{c}")
                     for c in range(DC)]
            hid_dram = dram.tile([2 * H, P, N], BF)

            def psE():
                return ps.tile([P, N], F32, tag="psE", name="pt", bufs=2)

            # ---- stage 1: projections (feature-major), naturals via XBAR ----
            # Only eo chunks 0,1 are computed up front (enough for heads
            # 0..3); the rest are emitted into the early head iterations'
            # PE bubbles via proj_pair().
            def proj_pair(eo, use_psav=False):
                for wT_sb, dstT, dstN in ((wqT, QT, Qn), (wkT, KT, Kn)):
                    if use_psav:
                        # the psav ring is idle until atq_part(0); using it
                        # decouples these fillers from the ACT-bound psE ring
                        pt = ps.tile([P, NC, 2 * Z], F32, tag="psav",
                                     name="pt", bufs=2)[:].rearrange(
                                         "p a b -> p (a b)")
                    else:
                        pt = psE()
                    for nh in range(2):
                        for do in range(DC):
                            nc.tensor.matmul(
                                pt[:, nh * 512:(nh + 1) * 512],
                                wT_sb[:, do, eo * P:(eo + 1) * P],
                                xT[:, do, nh * 512:(nh + 1) * 512],
                                start=(do == 0), stop=(do == DC - 1),
                            )
                    nc.vector.tensor_copy(dstT[:, eo, :], pt[:]
                                          if not use_psav else pt)
                    # natural layout via PE transposes (keeps the DMA device
                    # free in the congested early window)
                    if use_psav:
                        ptb = ps.tile([P, NC, 2 * Z], F32, tag="psav",
                                      name="ptb", bufs=2)[:].rearrange(
                                          "p a b -> p (a b)").bitcast(BF)
                    else:
                        ptb = psE()[:].bitcast(BF)
                    for qo in range(NC):
                        nc.tensor.transpose(
                            ptb[:, qo * P:(qo + 1) * P],
                            dstT[:, eo, qo * P:(qo + 1) * P], ident[:])
                    nc.vector.tensor_copy(
                        dstN[:, :, eo * P:(eo + 1) * P],
                        ptb[:, 0:NC * P].rearrange("p (a b) -> p a b", b=P))

            # eo=0 with tensor/nh interleave matched to input-load arrival
            # order (wqT0, xT_h0, wkT0, xT_h1): no PE queue-head blocking
            pts0 = {}
            for wT_sb, dstT, key in ((wqT, QT, "q"), (wkT, KT, "k")):
                pts0[key] = psE()
            # first QT quarter only needs the first xT quarter-load
    
            for seg0, seg1 in ((0, 256), (256, 512), (512, 1024)):
                for wT_sb, dstT, key in ((wqT, QT, "q"), (wkT, KT, "k")):
                    pt = pts0[key]
                    for do in range(DC):
                        nc.tensor.matmul(
                            pt[:, seg0:seg1],
                            wT_sb[:, do, 0:P],
                            xT[:, do, seg0:seg1],
                            start=(do == 0), stop=(do == DC - 1),
                        )
            for wT_sb, dstT, key in ((wqT, QT, "q"), (wkT, KT, "k")):
                dstN = Qn if key == "q" else Kn
                nc.vector.tensor_copy(dstT[:, 0, :], pts0[key][:])
                ptb = psE()[:].bitcast(BF)
                for qo in range(NC):
                    nc.tensor.transpose(
                        ptb[:, qo * P:(qo + 1) * P],
                        dstT[:, 0, qo * P:(qo + 1) * P], ident[:])
                nc.vector.tensor_copy(
                    dstN[:, :, 0:P],
                    ptb[:, 0:NC * P].rearrange("p (a b) -> p a b", b=P))

            # ---- stage 2+3: MLP layer 1 interleaved with per-head attention --
            # mlp1 chunk ho: hid rows [ho*128, (ho+1)*128) = relu(Wm x^T);
            # emitted as a list of closures so PE work can be interleaved
            # between S matmuls at fine grain.
            def mlp1_emit(ho):
                """Returns (steps, finish): steps = 12 matmul closures."""
                if ho % 2 == 0:
                    wt = stream.tile([P, DC, 2 * P], BF, tag="wmT", name="wt",
                                     bufs=2)
                    nc.sync.dma_start(wt[:], wmT_v[:, :, ho * P:(ho + 2) * P])
                    mlp1_emit.wt = wt
                wt = mlp1_emit.wt
                woff = (ho % 2) * P
                hchunk = stream.tile([P, N], BF, tag="hchunk", name="hchunk",
                                     bufs=2)
                phs = [None]
                steps = []
                for nh in range(2):
                    for do in range(DC):
                        def step(nh=nh, do=do):
                            if nh == 0 and do == 0:
                                phs[0] = psE()
                            nc.tensor.matmul(
                                phs[0][:, nh * 512:(nh + 1) * 512],
                                wt[:, do, woff:woff + P],
                                xT[:, do, nh * 512:(nh + 1) * 512],
                                start=(do == 0), stop=(do == DC - 1),
                            )
                        steps.append(step)

                def finish():
                    nc.vector.tensor_scalar_max(hchunk[:], phs[0][:], 0.0)
                    nc.sync.dma_start(hid_dram[ho], hchunk[:])
                return steps, finish

            def s_exp_norm(h, filler, ET_t):
                """E_h = exp(beta_h Q_h K_h^T), normalized in place per qo and
                DMA-transposed into ET_t in qo pairs as rows complete.
                `filler` yields PE closures (mlp1 matmuls) interleaved between
                S matmuls. Per-qo normalization needs a per-qo reciprocal of
                the accumulated rowsum."""
                zo = (h % 2) * Z
                c = h // 2
                QT_h = QT[zo:zo + Z, c, :]
                KT_h = KT[zo:zo + Z, c, :]
                E = hd.tile([P, NC, N], BF, tag="E", name="E", bufs=2)
                r_col = hd.tile([P, NC], F32, tag="r_col", name="r_col", bufs=2)
                rc_inv = hd.tile([P, NC], F32, tag="rc_inv", name="rc_inv",
                                 bufs=2)
                for qo in range(NC):
                    pt = psE()
                    for kh in range(2):
                        nc.tensor.matmul(
                            pt[:, kh * 512:(kh + 1) * 512],
                            QT_h[:, qo * P:(qo + 1) * P],
                            KT_h[:, kh * 512:(kh + 1) * 512],
                            start=True, stop=True,
                        )
                    nc.scalar.activation(
                        E[:, qo, :], pt[:], Exp,
                        scale=betav[:, h:h + 1],
                        accum_out=r_col[:, qo:qo + 1])
                    nc.vector.reciprocal(rc_inv[:, qo:qo + 1],
                                         r_col[:, qo:qo + 1])
                    nc.vector.tensor_scalar_mul(
                        E[:, qo, :], E[:, qo, :], rc_inv[:, qo:qo + 1])
                    for _ in range(3):
                        f = next(filler, None)
                        if f is not None:
                            f()
                return E, rc_inv

            def st_mm(h, ET_t, filler):
                """Odd heads: A^T computed on PE as exp(beta*S^T) UNNORMALIZED
                (row scale folded into the AV psum afterwards), written into
                ET_t with swapped (ko,qo) mid indexing."""
                zo = (h % 2) * Z
                c = h // 2
                QT_h = QT[zo:zo + Z, c, :]
                KT_h = KT[zo:zo + Z, c, :]
                for ko in range(NC):
                    pt = psE()
                    for qh in range(2):
                        nc.tensor.matmul(
                            pt[:, qh * 512:(qh + 1) * 512],
                            KT_h[:, ko * P:(ko + 1) * P],
                            QT_h[:, qh * 512:(qh + 1) * 512],
                            start=True, stop=True,
                        )
                    nc.scalar.activation(
                        ET_t[:, ko * NC:(ko + 1) * NC, 0:P],
                        pt[:].rearrange("p (a b) -> p a b", b=P), Exp,
                        scale=betav[:, h:h + 1])
                    f = next(filler, None)
                    if f is not None:
                        f()

            def atq_part(h, E):
                """ATQn for head h (depends only on E)."""
                off = (h % 2) * Z
                if h % 2 == 0:
                    atq_part.pav = ps.tile([P, NC, 2 * Z], F32, tag="psav",
                                           name="pav", bufs=2)
                    atq_part.patq = ps.tile([P, NC, 2 * Z], F32, tag="psav",
                                            name="patq", bufs=2)
                patq = atq_part.patq
                for ko in range(NC):
                    for qo in range(NC):
                        nc.tensor.matmul(
                            patq[:, ko, off:off + Z],
                            E[:, qo, ko * P:(ko + 1) * P],
                            Qn[:, qo, h * Z:(h + 1) * Z],
                            start=(qo == 0), stop=(qo == NC - 1),
                        )

            def av_part(h, ET_t, rc_inv):
                """AVn for head h; odd heads read the PE-computed unnormalized
                A^T (swapped indexing) and scale the psum rows afterwards; at
                odd h evict the pair and DMA-transpose into AVT/ATQT."""
                c = h // 2
                off = (h % 2) * Z
                odd = h % 2 == 1
                mm_path = False
                pav, patq = atq_part.pav, atq_part.patq
                for qo in range(NC):
                    for ko in range(NC):
                        idx = (ko * NC + qo) if mm_path else (qo * NC + ko)
                        nc.tensor.matmul(
                            pav[:, qo, off:off + Z],
                            ET_t[:, idx, 0:P],
                            Kn[:, ko, h * Z:(h + 1) * Z],
                            start=(ko == 0), stop=(ko == NC - 1),
                        )
                if mm_path:
                    for qo in range(NC):
                        nc.vector.tensor_scalar_mul(
                            pav[:, qo, off:off + Z], pav[:, qo, off:off + Z],
                            rc_inv[:, qo:qo + 1])
                if odd:
                    An = hd.tile([P, NC, 2 * Z], BF, tag="An", name="An",
                                 bufs=1)
                    Aq = hd.tile([P, NC, 2 * Z], BF, tag="Aq", name="Aq",
                                 bufs=1)
                    for tgt, src, psrc in ((ATQTs[c], Aq, patq),
                                           (AVTs[c], An, pav)):
                        ptb = psE()[:].bitcast(BF)
                        nc.vector.tensor_copy(src[:, 0:4, :], psrc[:, 0:4, :])
                        for qo in range(4):
                            nc.tensor.transpose(
                                ptb[:, qo * P:(qo + 1) * P], src[:, qo, :],
                                ident[:])
                        nc.vector.tensor_copy(src[:, 4:NC, :], psrc[:, 4:NC, :])
                        for qo in range(4, NC):
                            nc.tensor.transpose(
                                ptb[:, qo * P:(qo + 1) * P], src[:, qo, :],
                                ident[:])
                        nc.vector.tensor_copy(tgt[:], ptb[:, 0:NC * P])

            # software pipeline per iteration h:
            #   ATQn(h-1) [E-only] -> S/exp/norm(h) with mlp fillers ->
            #   AVn(h-1) [A^T had a full S-phase to transpose] -> evicts ->
            #   issue transpose of E(h).
            prev = None
            pre4 = []
            for h in range(H + 1):
                if prev is not None and h < 4:
                    atq_part(h - 1, prev[0])
                E = None
                filler = iter(())
                if h < H:
                    s0, f0 = mlp1_emit(2 * h)
                    s1, f1 = mlp1_emit(2 * h + 1)
                    filler = iter(s0 + [f0] + s1 + [f1])
                    E, rc_inv = s_exp_norm(h, filler, None)
                    for step in filler:
                        step()
                    if h == 0:
                        # remaining projection chunks fill the early-head
                        # bubble where PE would wait on the exp/norm chain;
                        # the psav ring keeps them off the ACT-bound psE ring
                        proj_pair(1, use_psav=True)
                        proj_pair(2, use_psav=True)
                        proj_pair(3, use_psav=True)
                    elif h in (1, 2):
                        proj_pair(h + 3)
                if prev is not None:
                    if h >= 4:
                        # late heads have no proj fillers left: the ET-
                        # independent ATQ work covers part of the A^T wait
                        atq_part(h - 1, prev[0])
                    av_part(h - 1, prev[1], prev[2])
                if h < H:
                    ET_t = hd.tile([P, NC * NC, PT], BF, tag="ET", name="ET",
                                   bufs=1)
                    nc.sync.dma_start(ET_t[:, :, 0:P], E[:], transpose=True)
                    prev = (E, ET_t, rc_inv)
                    if dbg and h == 0:
                        nc.sync.dma_start(dbg_d["E0"].ap(), E[:])
                        nc.sync.dma_start(dbg_d["ET0"].ap(), ET_t[:, :, 0:P])
                if h == H - 1:
                    # prefetch the first stage-4 streams before the last
                    # av_part so PE has data the moment stage 4 starts
                    for ho in range(4):
                        wmc = stream.tile([P, D], BF, tag="wmc", name="wmc",
                                          bufs=4)
                        nc.sync.dma_start(wmc[:], wm_v[:, ho, :])
                        hc = stream.tile([P, N], BF, tag="hc", name="hc",
                                         bufs=4)
                        nc.sync.dma_start(hc[:], hid_dram[ho])
                        pre4.append((wmc, hc))

            if dbg:
                nc.sync.dma_start(dbg_d["Qn"].ap(), Qn[:])
                nc.sync.dma_start(dbg_d["Kn"].ap(), Kn[:])
                nc.sync.dma_start(dbg_d["QT"].ap(), QT[:])
                avt_v = dbg_d["AVT"].ap()
                atqt_v = dbg_d["ATQT"].ap()
                for c in range(DC):
                    nc.sync.dma_start(avt_v[:, c, :].rearrange(
                        "p (m q) -> p m q", q=P), AVTs[c][:])
                    nc.sync.dma_start(atqt_v[:, c, :].rearrange(
                        "p (m q) -> p m q", q=P), ATQTs[c][:])
                hidc = stream.tile([P, N], BF, tag="hchunk", name="hdbg", bufs=2)
                nc.sync.dma_start(hidc[:], hid_dram[0])
                nc.sync.dma_start(dbg_d["hid0"].ap(), hidc[:])

            # ---- stage 4: out = AVc @ Wq + ATQc @ Wk + hid @ Wm ----
            # attn matmuls first (AVT/ATQT are ready before hid), hid stream
            # accumulates after; output DMAed straight from PSUM.
            for r4, nos in enumerate(([0, 1, 2, 3], [4, 5, 6, 7])):
                pouts = []
                for i in range(2):
                    t = psE()
                    pouts.append((t[:, 0:512], t[:, 512:768]))
                for i in range(2):
                    t = ps.tile([P, NC, 2 * Z], F32, tag="psav", name="po",
                                bufs=2)
                    pouts.append((t[:, 0:4, :], t[:, 4:6, :]))
                for i, no in enumerate(nos):
                    lo, hi = pouts[i]
                    for c2 in range(DC):
                        for lhss, w_sb in ((AVTs, wq), (ATQTs, wk)):
                            first = (c2 == 0 and lhss is AVTs)
                            nc.tensor.matmul(
                                lo,
                                lhss[c2][:, no, :],
                                w_sb[:, c2, 0:512],
                                start=first, stop=False,
                            )
                            nc.tensor.matmul(
                                hi,
                                lhss[c2][:, no, :],
                                w_sb[:, c2, 512:768],
                                start=first, stop=False,
                            )
                for ho in range(2 * H):
                    if r4 == 0 and ho < len(pre4):
                        wmc, hc = pre4[ho]
                    else:
                        wmc = stream.tile([P, D], BF, tag="wmc", name="wmc", bufs=4)
                        nc.sync.dma_start(wmc[:], wm_v[:, ho, :])
                        hc = stream.tile([P, N], BF, tag="hc", name="hc", bufs=4)
                        nc.sync.dma_start(hc[:], hid_dram[ho])
                    for i, no in enumerate(nos):
                        lo, hi = pouts[i]
                        last = (ho == 2 * H - 1)
                        nc.tensor.matmul(
                            lo, hc[:, no * P:(no + 1) * P], wmc[:, 0:512],
                            start=False, stop=last,
                        )
                        nc.tensor.matmul(
                            hi, hc[:, no * P:(no + 1) * P], wmc[:, 512:768],
                            start=False, stop=last,
                        )
                for g in range(2):
                    osb = stream.tile([P, 2, D], BF, tag="osb", name="osb",
                                      bufs=2)
                    for j in range(2):
                        i = 2 * g + j
                        lo, hi = pouts[i]
                        if i % 2 == 0:
                            nc.vector.tensor_copy(osb[:, j, 0:512], lo)
                            nc.vector.tensor_copy(osb[:, j, 512:768], hi)
                        else:
                            nc.scalar.copy(osb[:, j, 0:512], lo)
                            nc.scalar.copy(osb[:, j, 512:768], hi)
                    nc.sync.dma_start(
                        out_v[:, nos[2 * g]:nos[2 * g] + 2, :], osb[:])

    nc.compile()
    return nc


def _prep(x, Wq, Wk, betas, W_mlp):
    x = np.asarray(x, dtype=np.float32)
    Wq = np.asarray(Wq, dtype=np.float32)
    Wk = np.asarray(Wk, dtype=np.float32)
    betas = np.asarray(betas, dtype=np.float32)
    W_mlp = np.asarray(W_mlp, dtype=np.float32)

    wq = np.ascontiguousarray(Wq).astype(NPBF)
    wk = np.ascontiguousarray(Wk).astype(NPBF)
    wqT = np.ascontiguousarray(Wq.T).astype(NPBF)
    wkT = np.ascontiguousarray(Wk.T).astype(NPBF)
    wm = np.ascontiguousarray(W_mlp).astype(NPBF)
    wmT = np.ascontiguousarray(W_mlp.T).astype(NPBF)
    betav = np.ascontiguousarray(
        np.broadcast_to(betas[None, :], (P, H))).astype(np.float32)
    ident = np.eye(P, dtype=np.float32).astype(NPBF)

    in_maps = []
    for b in range(B):
        xT = np.ascontiguousarray(x[b].T).astype(NPBF)
        in_maps.append({
            "xT": xT, "wqT": wqT, "wkT": wkT, "wq": wq, "wk": wk,
            "wmT": wmT, "wm": wm, "betav": betav, "ident": ident,
        })
    return in_maps


def kernel(x, Wq, Wk, betas, W_mlp, _trace=False):
    if "nc" not in _CACHE:
        _CACHE["nc"] = _build()
    nc = _CACHE["nc"]
    in_maps = _prep(x, Wq, Wk, betas, W_mlp)
    res = run_bass_kernel_spmd(nc, in_maps, core_ids=list(range(B)), trace=_trace)
    out = np.stack([res.results[b]["out"] for b in range(B)], axis=0)
    _CACHE["last_result"] = res
    return out.astype(np.float32)
